# revision 1
# baseline (speedup 1.0000x reference)
"""Trainium2 Bass kernel for a 2-layer GCN + FC head (nn_CNNGNNModel).

Reference computation (PyG GCNConv semantics, symmetric normalization with
self-loops):
    deg[i]  = in-degree(i) + 1 ;  dinv = deg^-0.5
    A_hat   = D^-1/2 (A + I) D^-1/2   (aggregation by destination)
    h1 = relu(A_hat @ (x @ W1) + b1)
    h2 = relu(A_hat @ (h1 @ W2) + b2)
    out = h2 @ Wfc + bfc

Key algebraic trick: the per-edge weight dinv[src]*dinv[dst] is separable, so
we scale node features by dinv on the way out of each matmul (source side) and
scale the aggregate by dinv after the segment sum (dest side).  Message
passing then becomes a pure gather + segmented sum.

Distribution (8 NeuronCores, SPMD single program):
  - Nodes are sharded by id range: core c owns dests [c*12500, (c+1)*12500),
    padded to 12544 = 98*128 slots.  Within a core, dests are permuted
    (degree/chunk-count lexsort) to minimize gather padding; all index
    plumbing is precomputed on the host in "position" space
    pos = owner*12544 + slot.
  - Each layer: local matmul of the core's node block -> dinv-scaled bf16
    features -> AllGather to a full table [100352, 256] bf16 -> per-block
    dma_gather of incoming messages -> DVE fold-tree segmented sum -> relu.
  - dma_gather indices are int16 (<= 32767); the table is addressed in 4
    residue "chunks" of 25088 rows via the in_ AP base offset, so each block
    issues up to 4 gather calls (one per chunk) padded per-(block,chunk) to
    the max count over the 128 dest slots of the block (zero-row padding).
  - Weights are replicated; output [12544, 1000] f32 per core is reassembled
    (inverse permutation) on the host.
"""

import numpy as np
import ml_dtypes

import concourse.bass as bass
import concourse.bacc as bacc
import concourse.mybir as mybir
import concourse.tile as tile
from concourse.bass_utils import run_bass_kernel_spmd
from concourse.masks import make_identity

BF16 = ml_dtypes.bfloat16

N_CORES = 8
N_NODES = 100000
IN_DIM = 512
HID = 256
NCLS = 1000
NLOC = 12500          # real dests per core
SLOTS = 12544         # padded dests per core (98 blocks of 128)
BLOCKS = SLOTS // 128  # 98
NCHUNK = 4
CH_ROWS = 2 * SLOTS   # 25088 rows per chunk (= 2 cores)
ZERO_LOCAL = 12543    # chunk-local row guaranteed to be a zero pad row
LCAP = 56             # max gather-tile columns per round
P = 128


def _wrap_idx(flat_idx: np.ndarray) -> np.ndarray:
    """Wrap a flat int16 index array [n] (n % 16 == 0) into the dma_gather
    SBUF layout [128, n//16]: position j -> (partition j%16, column j//16),
    replicated across the eight 16-partition bands."""
    n = flat_idx.shape[0]
    band = flat_idx.reshape(n // 16, 16).T  # [16, n//16]
    return np.tile(band, (8, 1)).astype(np.int16)


def _preprocess(x, edge_index, W1, b1, W2, b2, Wfc, bfc):
    """All host-side graph preprocessing. Returns (plan, in_maps, ids_order)."""
    row = np.asarray(edge_index[0], dtype=np.int64)
    col = np.asarray(edge_index[1], dtype=np.int64)

    deg = np.bincount(col, minlength=N_NODES).astype(np.int64) + 1
    dinv = (1.0 / np.sqrt(deg.astype(np.float32))).astype(np.float32)

    # --- node -> (core, slot) assignment, built to minimize gather padding.
    # A node's "chunk" as a message SOURCE is core//2 (4 chunks of 2 cores,
    # 25088 table rows each; dma_gather int16 indices only reach 32767 rows,
    # hence the chunked gather).  We greedily color nodes into the 4 chunks
    # so that every dest's in-edges are spread evenly over chunks; then the
    # per-(block,chunk) max padding is small.  Within a chunk, dests are
    # sorted by their count vector and striped across the chunk's 2 cores.
    rng = np.random.default_rng(12345)
    all_src0 = np.concatenate([row, np.arange(N_NODES)])
    all_dst0 = np.concatenate([col, np.arange(N_NODES)])
    o = np.argsort(all_src0, kind="stable")
    sr = all_src0[o]
    sc = all_dst0[o]
    starts = np.searchsorted(sr, np.arange(N_NODES + 1))
    deg_out = np.diff(starts)
    target = deg.astype(np.float32) / NCHUNK

    CAP = CH_ROWS - 64  # leave pad rows in every chunk
    color = np.full(N_NODES, -1, np.int8)
    kmat = np.zeros((N_NODES, NCHUNK), np.int32)
    sizes = np.zeros(NCHUNK, np.int64)
    order_src = rng.permutation(N_NODES)
    B = 1000
    for i in range(0, N_NODES, B):
        batch = order_src[i:i + B]
        reps = deg_out[batch]
        idx = np.concatenate(
            [np.arange(starts[s], starts[s + 1]) for s in batch]
        )
        dsts = sc[idx]
        srcrep = np.repeat(np.arange(len(batch)), reps)
        dev = kmat[dsts].astype(np.float32) - target[dsts][:, None]
        score = np.zeros((len(batch), NCHUNK), np.float32)
        np.add.at(score, srcrep, dev)
        score += (sizes / CAP).astype(np.float32) * 0.5 * reps[:, None]
        score[:, sizes >= CAP] = 1e18
        ch = score.argmin(1).astype(np.int8)
        color[batch] = ch
        np.add.at(sizes, ch, 1)
        np.add.at(kmat, (dsts, ch[srcrep]), 1)

    slot = np.empty(N_NODES, np.int64)
    core_of = np.empty(N_NODES, np.int64)
    ids_order = []
    n_core = [0] * N_CORES
    for q in range(NCHUNK):
        nodes_q = np.where(color == q)[0]
        kk = kmat[nodes_q]
        mm = kk.max(1)
        o2 = np.lexsort((-kk[:, 3], -kk[:, 2], -kk[:, 1], -kk[:, 0], -mm))
        nq = nodes_q[o2]
        r = np.arange(len(nq))
        core_of[nq] = 2 * q + (r % 2)
        slot[nq] = (r // 256) * P + (r % 256) // 2
    pos = core_of * SLOTS + slot
    for c in range(N_CORES):
        ids = np.where(core_of == c)[0]
        ids = ids[np.argsort(slot[ids])]
        ids_order.append(ids)
        n_core[c] = len(ids)
        assert np.array_equal(slot[ids], np.arange(len(ids)))

    # --- edge lists sorted by (dest position, src chunk)
    # include self loops as edges
    all_src = np.concatenate([row, np.arange(N_NODES)])
    all_dst = np.concatenate([col, np.arange(N_NODES)])
    dst_pos = pos[all_dst]
    src_pos = pos[all_src]
    s_chunk = src_pos // CH_ROWS
    order = np.lexsort((s_chunk, dst_pos))
    dst_pos = dst_pos[order]
    src_pos = src_pos[order]
    s_chunk = s_chunk[order]

    # per (dest position, chunk) counts and CSR starts
    key = dst_pos * NCHUNK + s_chunk
    kcnt = np.bincount(key, minlength=N_CORES * SLOTS * NCHUNK).reshape(
        N_CORES, SLOTS, NCHUNK
    )
    csr = np.zeros(N_CORES * SLOTS * NCHUNK + 1, np.int64)
    np.cumsum(kcnt.ravel(), out=csr[1:])

    # per-(block, chunk) K shared across all cores
    kblk = kcnt.reshape(N_CORES, BLOCKS, P, NCHUNK)
    Kbq = kblk.max(axis=(0, 2))  # [BLOCKS, NCHUNK]

    real_edges = int(kcnt.sum())
    padded_edges = int(Kbq.sum() * P * N_CORES)
    plan_inflation = padded_edges / real_edges

    # round packing: per block, greedily pack chunks into rounds of <= LCAP cols
    rounds = []  # list per block: list of rounds, each = list of (q, Kq, Cq_in_round)
    for b in range(BLOCKS):
        rs = []
        cur = []
        cur_cols = 0
        for q in range(NCHUNK):
            kq = int(Kbq[b, q])
            if kq == 0:
                continue
            take = 0
            while take < kq:
                room = LCAP - cur_cols
                if room == 0:
                    rs.append(cur)
                    cur = []
                    cur_cols = 0
                    room = LCAP
                # SWDGE descriptor-ring limit: dma_gather crashes above
                # ~1024 indices per call -> cap each call at 8 columns.
                part = min(room, kq - take, 8)
                cur.append((q, take, part, cur_cols))
                cur_cols += part
                take += part
        if cur:
            rs.append(cur)
        rounds.append(rs)

    # --- build per-core index arrays (shared shapes; values differ)
    # also build layer-agnostic gather call plan with compile-time offsets
    idx_arrays = [[] for _ in range(N_CORES)]
    call_plan = []  # per block: list of rounds: list of (q, ncols, round_col, idx_off16)
    off16 = 0
    p_ar = np.arange(P)
    for b in range(BLOCKS):
        blk_plan = []
        for rs in rounds[b]:
            r_plan = []
            for (q, take0, ncols, rcol) in rs:
                n_idx = P * ncols
                r_plan.append((q, ncols, rcol, off16))
                off16 += P * (n_idx // 16)  # wrapped int16 elements: 128 * n/16
                for c in range(N_CORES):
                    slots_g = c * SLOTS + b * P + p_ar  # global dest rows
                    base = csr[(slots_g * NCHUNK + q)]
                    kreal = kcnt[c, b * P + p_ar, q]
                    # position j = col*128 + p ; col in [take0, take0+ncols)
                    jj = take0 + np.arange(ncols)
                    # idx [ncols, P]
                    gather_rows = np.full((ncols, P), q * CH_ROWS + (q * 2) * SLOTS,
                                          np.int64)
                    valid = jj[:, None] < kreal[None, :]
                    src_take = np.minimum(jj[:, None], kreal[None, :] - 1)
                    rowsel = src_pos[base[None, :] + src_take]
                    zero_row = q * CH_ROWS + ZERO_LOCAL  # global pos of a 0 row
                    gather_rows = np.where(valid, rowsel, zero_row)
                    local = (gather_rows - q * CH_ROWS).astype(np.int16)
                    idx_arrays[c].append(_wrap_idx(local.reshape(-1)))
            blk_plan.append(r_plan)
        call_plan.append(blk_plan)

    idx_in = [np.concatenate([a.reshape(-1) for a in idx_arrays[c]])
              for c in range(N_CORES)]
    assert idx_in[0].shape[0] == off16

    # --- per-core dense inputs
    xb = np.ascontiguousarray(x).astype(BF16)
    in_maps = []
    w1_in = np.ascontiguousarray(
        W1.astype(BF16).reshape(NCHUNK, P, HID).transpose(1, 0, 2).reshape(P, NCHUNK * HID)
    )
    w2_in = np.ascontiguousarray(
        W2.astype(BF16).reshape(2, P, HID).transpose(1, 0, 2).reshape(P, 2 * HID)
    )
    wfc_in = np.ascontiguousarray(
        Wfc.astype(BF16).reshape(2, P, NCLS).transpose(1, 0, 2).reshape(P, 2 * NCLS)
    )
    has_b1 = bool(np.any(b1)) ; has_b2 = bool(np.any(b2)) ; has_bfc = bool(np.any(bfc))
    b1_in = np.tile(np.asarray(b1, np.float32)[None, :], (P, 1))
    b2_in = np.tile(np.asarray(b2, np.float32)[None, :], (P, 1))
    bfc_in = np.tile(np.asarray(bfc, np.float32)[None, :], (P, 1))

    for c in range(N_CORES):
        A = np.zeros((SLOTS, IN_DIM), BF16)
        A[:n_core[c]] = xb[ids_order[c]]
        xtt = np.ascontiguousarray(
            A.reshape(BLOCKS, P, NCHUNK, P).transpose(0, 3, 2, 1).reshape(BLOCKS, P, IN_DIM)
        )
        dv = np.ones(SLOTS, np.float32)
        dv[:n_core[c]] = dinv[ids_order[c]]
        dvp = np.ascontiguousarray(dv.reshape(BLOCKS, P).T)  # [128, 98]
        m = {
            "xtt": xtt,
            "dinvp": dvp,
            "idxs": idx_in[c],
            "w1": w1_in,
            "w2": w2_in,
            "wfc": wfc_in,
        }
        if has_b1:
            m["b1b"] = b1_in
        if has_b2:
            m["b2b"] = b2_in
        if has_bfc:
            m["bfcb"] = bfc_in
        in_maps.append(m)

    plan = {
        "call_plan": call_plan,
        "rounds_cols": [[sum(p[1] for p in r) for r in blk] for blk in call_plan],
        "idx_total": off16,
        "has_b1": has_b1,
        "has_b2": has_b2,
        "has_bfc": has_bfc,
        "inflation": plan_inflation,
        "n_core": n_core,
    }
    return plan, in_maps, ids_order


def _build_program(plan, sim_single_core=False, stop_after="full"):
    """Build the SPMD Bass program (one program, all cores).

    stop_after: one of "mm1", "ag1", "g1", "mm2", "ag2", "g2", "full" —
    truncates the program after that phase (for bisection/debug)."""
    STAGES = ["mm1", "ag1", "g1a", "g1b", "g1c", "g1", "mm2", "ag2", "g2", "full"]
    stop_idx = STAGES.index(stop_after)
    detail = {"g1a": 1, "g1b": 2, "g1c": 3}.get(stop_after, 4)
    nc = bacc.Bacc("TRN2", target_bir_lowering=False, debug=False,
                   num_devices=N_CORES)
    dt = mybir.dt

    xtt = nc.dram_tensor("xtt", [BLOCKS, P, IN_DIM], dt.bfloat16, kind="ExternalInput")
    dinvp = nc.dram_tensor("dinvp", [P, BLOCKS], dt.float32, kind="ExternalInput")
    idxs = nc.dram_tensor("idxs", [plan["idx_total"]], dt.int16, kind="ExternalInput")
    w1 = nc.dram_tensor("w1", [P, NCHUNK * HID], dt.bfloat16, kind="ExternalInput")
    w2 = nc.dram_tensor("w2", [P, 2 * HID], dt.bfloat16, kind="ExternalInput")
    wfc = nc.dram_tensor("wfc", [P, 2 * NCLS], dt.bfloat16, kind="ExternalInput")
    b1b = (nc.dram_tensor("b1b", [P, HID], dt.float32, kind="ExternalInput")
           if plan["has_b1"] else None)
    b2b = (nc.dram_tensor("b2b", [P, HID], dt.float32, kind="ExternalInput")
           if plan["has_b2"] else None)
    bfcb = (nc.dram_tensor("bfcb", [P, NCLS], dt.float32, kind="ExternalInput")
            if plan["has_bfc"] else None)
    out = nc.dram_tensor("out", [SLOTS, NCLS], dt.float32, kind="ExternalOutput")

    hloc1 = nc.dram_tensor("hloc1", [SLOTS, HID], dt.bfloat16)
    hloc2 = nc.dram_tensor("hloc2", [SLOTS, HID], dt.bfloat16)
    hfull1 = nc.dram_tensor("hfull1", [N_CORES * SLOTS, HID], dt.bfloat16,
                            addr_space="Shared")
    hfull2 = nc.dram_tensor("hfull2", [N_CORES * SLOTS, HID], dt.bfloat16,
                            addr_space="Shared")
    h1T = nc.dram_tensor("h1T", [BLOCKS, P, HID], dt.bfloat16)
    h2T = nc.dram_tensor("h2T", [BLOCKS, P, HID], dt.bfloat16)

    call_plan = plan["call_plan"]
    rounds_cols = plan["rounds_cols"]

    with tile.TileContext(nc) as tc:
        with (
            tc.tile_pool(name="const", bufs=1) as constp,
            tc.tile_pool(name="xt", bufs=3) as xtp,
            tc.tile_pool(name="hl", bufs=3) as hlp,
            tc.tile_pool(name="idx", bufs=6) as idxp,
            tc.tile_pool(name="g", bufs=3) as gp,
            tc.tile_pool(name="hsmall", bufs=4) as hsp,
            tc.tile_pool(name="fco", bufs=2) as fcop,
            tc.tile_pool(name="mmps", bufs=2, space="PSUM") as mmps,
            tc.tile_pool(name="tpps", bufs=2, space="PSUM") as tpps,
            tc.tile_pool(name="fcps", bufs=2, space="PSUM") as fcps,
        ):
            # resident constants
            w1_sb = constp.tile([P, NCHUNK * HID], dt.bfloat16)
            nc.sync.dma_start(out=w1_sb[:], in_=w1[:])
            w2_sb = constp.tile([P, 2 * HID], dt.bfloat16)
            nc.sync.dma_start(out=w2_sb[:], in_=w2[:])
            wfc_sb = constp.tile([P, 2 * NCLS], dt.bfloat16)
            nc.sync.dma_start(out=wfc_sb[:], in_=wfc[:])
            dv_sb = constp.tile([P, BLOCKS], dt.float32)
            nc.sync.dma_start(out=dv_sb[:], in_=dinvp[:])
            ident = constp.tile([P, P], dt.bfloat16)
            make_identity(nc, ident[:])
            b1_sb = b2_sb = bfc_sb = None
            if b1b is not None:
                b1_sb = constp.tile([P, HID], dt.float32)
                nc.sync.dma_start(out=b1_sb[:], in_=b1b[:])
            if b2b is not None:
                b2_sb = constp.tile([P, HID], dt.float32)
                nc.sync.dma_start(out=b2_sb[:], in_=b2b[:])
            if bfcb is not None:
                bfc_sb = constp.tile([P, NCLS], dt.float32)
                nc.sync.dma_start(out=bfc_sb[:], in_=bfcb[:])

            def layer_matmul(src_dram, w_sb, nk, hloc):
                """hloc[mb] = dinv * (A @ W) as bf16, A tiles from src_dram."""
                for mb in range(BLOCKS):
                    at = xtp.tile([P, nk * P], dt.bfloat16, tag="xt")
                    nc.sync.dma_start(out=at[:], in_=src_dram[mb])
                    ps = mmps.tile([P, HID], dt.float32, space="PSUM", tag="mm")
                    for k in range(nk):
                        nc.tensor.matmul(
                            out=ps[:],
                            lhsT=at[:, k * P:(k + 1) * P],
                            rhs=w_sb[:, k * HID:(k + 1) * HID],
                            start=(k == 0),
                            stop=(k == nk - 1),
                        )
                    hl = hlp.tile([P, HID], dt.bfloat16, tag="hl")
                    nc.scalar.activation(
                        out=hl[:], in_=ps[:],
                        func=mybir.ActivationFunctionType.Copy,
                        scale=dv_sb[:, mb:mb + 1],
                    )
                    nc.sync.dma_start(out=hloc[mb * P:(mb + 1) * P, :], in_=hl[:])

            def all_gather(hloc, hfull):
                if sim_single_core:
                    nc.sync.dma_start(out=hfull[0:SLOTS, :], in_=hloc[:])
                else:
                    nc.gpsimd.collective_compute(
                        "AllGather",
                        mybir.AluOpType.bypass,
                        replica_groups=[list(range(N_CORES))],
                        ins=[hloc[:]],
                        outs=[hfull[:]],
                    )

            def gather_layer(hfull, b_sb, hT, detail=4):
                """h = relu(dinv * segsum(gather(hfull))) (+bias);
                writes transposed tiles to hT."""
                for b in range(BLOCKS):
                    partials = []
                    for r, r_plan in enumerate(call_plan[b]):
                        ncols = rounds_cols[b][r]
                        g = gp.tile([P, LCAP * HID], dt.bfloat16, tag="g")
                        for (q, kq, rcol, ioff) in r_plan:
                            it = idxp.tile([P, LCAP * 8], dt.int16, tag="idx")
                            n16 = P * kq * 8
                            nc.sync.dma_start(
                                out=it[:, :kq * 8],
                                in_=idxs[ioff:ioff + n16].rearrange(
                                    "(p s) -> p s", p=P),
                            )
                            nidx = P * kq
                            nc.gpsimd.dma_gather(
                                g[:, rcol * HID:(rcol + kq) * HID].rearrange(
                                    "p (l d) -> p l d", d=HID),
                                hfull[q * CH_ROWS:(q + 1) * CH_ROWS, :],
                                it[:, :kq * 8],
                                nidx,
                                nidx,
                                HID,
                            )
                        # fold tree over ncols columns
                        cur = ncols if detail >= 2 else 1
                        while cur > 1:
                            half = cur // 2
                            keep = cur - half
                            nc.vector.tensor_tensor(
                                out=g[:, 0:half * HID],
                                in0=g[:, 0:half * HID],
                                in1=g[:, keep * HID:(keep + half) * HID],
                                op=mybir.AluOpType.add,
                            )
                            cur = keep
                        partials.append(g)
                    for r in range(1, len(partials)):
                        nc.vector.tensor_tensor(
                            out=partials[0][:, 0:HID],
                            in0=partials[0][:, 0:HID],
                            in1=partials[r][:, 0:HID],
                            op=mybir.AluOpType.add,
                        )
                    agg = partials[0]
                    if detail < 3:
                        continue
                    h = hsp.tile([P, HID], dt.bfloat16, tag="h")
                    if b_sb is not None:
                        hf = hsp.tile([P, HID], dt.float32, tag="hf")
                        nc.vector.tensor_scalar(
                            out=hf[:], in0=agg[:, 0:HID],
                            scalar1=dv_sb[:, b:b + 1], scalar2=None,
                            op0=mybir.AluOpType.mult,
                        )
                        nc.vector.tensor_tensor(
                            out=hf[:], in0=hf[:], in1=b_sb[:],
                            op=mybir.AluOpType.add,
                        )
                        nc.scalar.activation(
                            out=h[:], in_=hf[:],
                            func=mybir.ActivationFunctionType.Relu,
                        )
                    else:
                        nc.scalar.activation(
                            out=h[:], in_=agg[:, 0:HID],
                            func=mybir.ActivationFunctionType.Relu,
                            scale=dv_sb[:, b:b + 1],
                        )
                    if detail < 4:
                        continue
                    # transpose h [128 nodes, 256 feat] -> hT[mb] [128 feat-part, 256]
                    ht = hsp.tile([P, HID], dt.bfloat16, tag="ht")
                    for k in range(2):
                        tp = tpps.tile([P, P], dt.bfloat16, space="PSUM", tag="tp")
                        nc.tensor.transpose(
                            out=tp[:], in_=h[:, k * P:(k + 1) * P], identity=ident[:]
                        )
                        nc.scalar.copy(out=ht[:, k * P:(k + 1) * P], in_=tp[:])
                    nc.sync.dma_start(out=hT[b], in_=ht[:])

            # ---- layer 1
            layer_matmul(xtt, w1_sb, NCHUNK, hloc1)
            if stop_idx >= 1:
                all_gather(hloc1, hfull1)
            if stop_idx >= 2:
                gather_layer(hfull1, b1_sb, h1T, detail=detail)
            # ---- layer 2
            if stop_idx >= 6:
                layer_matmul(h1T, w2_sb, 2, hloc2)
            if stop_idx >= 7:
                all_gather(hloc2, hfull2)
            if stop_idx >= 8:
                gather_layer(hfull2, b2_sb, h2T)
            # ---- FC head
            for mb in range(BLOCKS if stop_idx >= 9 else 0):
                at = xtp.tile([P, 2 * P], dt.bfloat16, tag="xt")
                nc.sync.dma_start(out=at[:], in_=h2T[mb])
                fo = fcop.tile([P, NCLS], dt.float32, tag="fco")
                for n in range(2):
                    ps = fcps.tile([P, NCLS // 2], dt.float32, space="PSUM", tag="fc")
                    for k in range(2):
                        nc.tensor.matmul(
                            out=ps[:],
                            lhsT=at[:, k * P:(k + 1) * P],
                            rhs=wfc_sb[:, k * NCLS + n * (NCLS // 2):
                                       k * NCLS + (n + 1) * (NCLS // 2)],
                            start=(k == 0),
                            stop=(k == 1),
                        )
                        pass
                    nc.vector.tensor_copy(
                        out=fo[:, n * (NCLS // 2):(n + 1) * (NCLS // 2)], in_=ps[:]
                    )
                if bfc_sb is not None:
                    nc.vector.tensor_tensor(
                        out=fo[:], in0=fo[:], in1=bfc_sb[:],
                        op=mybir.AluOpType.add,
                    )
                nc.sync.dma_start(out=out[mb * P:(mb + 1) * P, :], in_=fo[:])

    nc.compile()
    return nc


_CACHE = {}


def kernel(x, edge_index, W1, b1, W2, b2, Wfc, bfc):
    x = np.asarray(x)
    plan, in_maps, ids_order = _preprocess(x, edge_index, W1, b1, W2, b2, Wfc, bfc)
    nc = _build_program(plan)
    res = run_bass_kernel_spmd(nc, in_maps, core_ids=list(range(N_CORES)))
    full = np.empty((N_NODES, NCLS), np.float32)
    for c in range(N_CORES):
        full[ids_order[c]] = res.results[c]["out"][: len(ids_order[c])]
    return full



# revision 21
# speedup vs baseline: 1.3813x; 1.3813x over previous
"""Trainium2 Bass kernel for a 2-layer GCN + FC head (nn_CNNGNNModel).

Reference computation (PyG GCNConv semantics, symmetric normalization with
self-loops):
    deg[i]  = in-degree(i) + 1 ;  dinv = deg^-0.5
    A_hat   = D^-1/2 (A + I) D^-1/2   (aggregation by destination)
    h1 = relu(A_hat @ (x @ W1) + b1)
    h2 = relu(A_hat @ (h1 @ W2) + b2)
    out = h2 @ Wfc + bfc

Key algebra: the per-edge weight dinv[src]*dinv[dst] is separable.  The
kernel keeps per-node "table" rows t_l = dinv * (input_l @ W_l); message
passing is a pure gather + sum of table rows, and the layer output fed to
the next matmul is relu(dinv^2 * agg + dinv * b)  (= dinv * relu(dinv*agg
+ b), folded into one activation since dinv > 0).  x is pre-scaled by dinv
on the host so mm1 needs no per-row scale.  Self-loop messages are exactly
the local table rows, kept resident in SBUF and added directly - they are
excluded from the gather.

Distribution (8 NeuronCores, SPMD single program):
  - Nodes sharded by id range: core c owns dests [c*12500, (c+1)*12500),
    padded to 12544 = 98*128 slots.  Within a core, dests are permuted
    (chunk-count lexsort) to minimize gather padding; index plumbing is
    host-precomputed in "position" space pos = owner*12544 + slot.
  - Each layer: local matmul -> bf16 table rows -> AllGather to a full
    table [100352, 256] -> per-block dma_gather of incoming messages ->
    segmented sum on the TensorEngine (identity-matmul accumulation into
    PSUM; one matmul per gathered 128-message column, Ldweights-free) ->
    activation (PSUM -> SBUF).
  - dma_gather indices are int16 (<= 32767); the table is addressed in 4
    residue chunks of 25088 rows via the in_ AP base offset.  Each chunk's
    gather columns form one stream across blocks, cut into fixed 8-column
    calls (1024 descriptors - the SWDGE per-call limit), so the 994 ns
    per-call descriptor-generation overhead stays below the DMA transfer
    time and pipelines behind it.  Columns are padded per-(block,chunk) to
    the max count over the 128 dest slots (zero-row padding).  Indices are
    stored pre-wrapped in DRAM as [128, ncols*8] int16 in call-emission
    order and DMA'd in 40-call slabs.
  - mm2 / FC head are fused per-block into the gather phases (transpose
    via PE + matmul), so no transposed intermediate hits DRAM.  mm1 input
    tiles and hloc writes are batched 7 blocks per DMA (HWDGE relief).
    The output is written bf16 and upcast to fp32 on the host.
  - Weights are replicated; output [12544, 1000] per core is reassembled
    (inverse permutation) on the host.
"""

import numpy as np
import ml_dtypes

import concourse.bass as bass
import concourse.bacc as bacc
import concourse.mybir as mybir
import concourse.tile as tile
from concourse.bass_utils import run_bass_kernel_spmd
from concourse.masks import make_identity

BF16 = ml_dtypes.bfloat16

N_CORES = 8
N_NODES = 100000
IN_DIM = 512
HID = 256
NCLS = 1000
NLOC = 12500          # real dests per core
SLOTS = 12544         # padded dests per core (98 blocks of 128)
BLOCKS = SLOTS // 128  # 98
NCHUNK = 4
CH_ROWS = 2 * SLOTS   # 25088 rows per chunk (= 2 cores)
ZERO_LOCAL = 12543    # chunk-local row guaranteed to be a zero pad row
CALL_COLS = 8         # gather columns per call (1024 descriptors = the
                      # SWDGE per-call limit on the real runtime)
CALLS_PER_SLAB = 40   # index slab granularity (calls per slab DMA)
MMG = 7               # mm1 / hloc batching granularity (blocks per group)
P = 128


def _wrap_idx(flat_idx: np.ndarray) -> np.ndarray:
    """Wrap a flat int16 index array [n] (n % 16 == 0) into the dma_gather
    SBUF layout [128, n//16]: position j -> (partition j%16, column j//16),
    replicated across the eight 16-partition bands."""
    n = flat_idx.shape[0]
    band = flat_idx.reshape(n // 16, 16).T  # [16, n//16]
    return np.tile(band, (8, 1)).astype(np.int16)


def _preprocess(x, edge_index, W1, b1, W2, b2, Wfc, bfc):
    """All host-side graph preprocessing. Returns (plan, in_maps, ids_order)."""
    row = np.asarray(edge_index[0], dtype=np.int64)
    col = np.asarray(edge_index[1], dtype=np.int64)

    deg = np.bincount(col, minlength=N_NODES).astype(np.int64) + 1
    dinv = (1.0 / np.sqrt(deg.astype(np.float32))).astype(np.float32)

    # --- node -> (core, slot) assignment, built to minimize gather padding.
    # A node's "chunk" as a message SOURCE is core//2 (4 chunks of 2 cores,
    # 25088 table rows each; dma_gather int16 indices only reach 32767 rows,
    # hence the chunked gather).  We greedily color nodes into the 4 chunks
    # so that every dest's in-edges are spread evenly over chunks; then the
    # per-(block,chunk) max padding is small.  Within a chunk, dests are
    # sorted by their count vector and striped across the chunk's 2 cores.
    # Self-loops are handled locally (SBUF-resident add) and excluded here.
    rng = np.random.default_rng(12345)
    o = np.argsort(row, kind="stable")
    sr = row[o]
    sc = col[o]
    starts = np.searchsorted(sr, np.arange(N_NODES + 1))
    deg_out = np.diff(starts)
    target = (deg.astype(np.float32) - 1.0) / NCHUNK

    CAP = CH_ROWS - 64  # leave pad rows in every chunk
    color = np.full(N_NODES, -1, np.int8)
    kmat = np.zeros((N_NODES, NCHUNK), np.int32)
    sizes = np.zeros(NCHUNK, np.int64)
    order_src = rng.permutation(N_NODES)
    B = 1000
    for i in range(0, N_NODES, B):
        batch = order_src[i:i + B]
        reps = deg_out[batch]
        idx = np.concatenate(
            [np.arange(starts[s], starts[s + 1]) for s in batch]
        )
        dsts = sc[idx]
        srcrep = np.repeat(np.arange(len(batch)), reps)
        dev = kmat[dsts].astype(np.float32) - target[dsts][:, None]
        score = np.zeros((len(batch), NCHUNK), np.float32)
        np.add.at(score, srcrep, dev)
        score += (sizes / CAP).astype(np.float32) * 0.5 * reps[:, None]
        score[:, sizes >= CAP] = 1e18
        ch = score.argmin(1).astype(np.int8)
        color[batch] = ch
        np.add.at(sizes, ch, 1)
        np.add.at(kmat, (dsts, ch[srcrep]), 1)

    slot = np.empty(N_NODES, np.int64)
    core_of = np.empty(N_NODES, np.int64)
    ids_order = []
    n_core = [0] * N_CORES
    for q in range(NCHUNK):
        nodes_q = np.where(color == q)[0]
        kk = kmat[nodes_q]
        mm = kk.max(1)
        o2 = np.lexsort((-kk[:, 3], -kk[:, 2], -kk[:, 1], -kk[:, 0], -mm))
        nq = nodes_q[o2]
        r = np.arange(len(nq))
        core_of[nq] = 2 * q + (r % 2)
        slot[nq] = (r // 256) * P + (r % 256) // 2
    pos = core_of * SLOTS + slot
    for c in range(N_CORES):
        ids = np.where(core_of == c)[0]
        ids = ids[np.argsort(slot[ids])]
        ids_order.append(ids)
        n_core[c] = len(ids)
        assert np.array_equal(slot[ids], np.arange(len(ids)))

    # --- edge lists sorted by (dest position, src chunk); no self loops
    dst_pos = pos[col]
    src_pos = pos[row]
    s_chunk = src_pos // CH_ROWS
    order = np.lexsort((s_chunk, dst_pos))
    dst_pos = dst_pos[order]
    src_pos = src_pos[order]
    s_chunk = s_chunk[order]

    # per (dest position, chunk) counts and CSR starts
    key = dst_pos * NCHUNK + s_chunk
    kcnt = np.bincount(key, minlength=N_CORES * SLOTS * NCHUNK).reshape(
        N_CORES, SLOTS, NCHUNK
    )
    csr = np.zeros(N_CORES * SLOTS * NCHUNK + 1, np.int64)
    np.cumsum(kcnt.ravel(), out=csr[1:])

    # per-(block, chunk) K shared across all cores
    kblk = kcnt.reshape(N_CORES, BLOCKS, P, NCHUNK)
    Kbq = kblk.max(axis=(0, 2))  # [BLOCKS, NCHUNK]

    real_edges = int(kcnt.sum())
    padded_edges = int(Kbq.sum() * P * N_CORES)
    plan_inflation = padded_edges / real_edges

    # --- per-core index arrays (shared shapes; values differ), call plan.
    # Each chunk's gather columns form one stream across all blocks; the
    # stream is cut into fixed CALL_COLS-column calls (1024 descriptors),
    # independent of block boundaries.  Calls are emitted in the order the
    # per-block fold consumes them; indices are stored pre-wrapped in that
    # emission order as one [128, total_cols*8] int16 image per core,
    # loaded in CALLS_PER_SLAB-call slabs.
    p_ar = np.arange(P)

    def call_cols(b, q, j0, n):
        """Index columns j0..j0+n of (block b, chunk q) for all cores ->
        [N_CORES][128, n*8] wrapped int16."""
        outs = []
        jj = j0 + np.arange(n)
        zero_row = ZERO_LOCAL
        for c in range(N_CORES):
            slots_g = c * SLOTS + b * P + p_ar
            base = csr[(slots_g * NCHUNK + q)]
            kreal = kcnt[c, b * P + p_ar, q]
            valid = jj[:, None] < kreal[None, :]
            src_take = np.minimum(jj[:, None], kreal[None, :] - 1)
            rowsel = src_pos[base[None, :] + src_take]
            gather_rows = np.where(valid, rowsel, q * CH_ROWS + zero_row)
            local = (gather_rows - q * CH_ROWS).astype(np.int16)
            outs.append(_wrap_idx(local.reshape(-1)))
        return outs

    stream_total = Kbq.sum(axis=0)          # cols per chunk stream
    blk_start = np.zeros((BLOCKS, NCHUNK), np.int64)
    blk_start[1:] = np.cumsum(Kbq[:-1], axis=0)

    # emission order: walk blocks; per chunk issue stream calls as needed
    idx_cols = [[] for _ in range(N_CORES)]
    calls = []        # (q, ncols, img_off)  in emission order
    block_first_call = []  # per block: number of calls issued before it
    issued_cols = [0] * NCHUNK   # stream cols already covered by calls
    img_off = 0
    for b in range(BLOCKS):
        block_first_call.append(len(calls))
        for q in range(NCHUNK):
            need = int(blk_start[b, q] + Kbq[b, q])
            while issued_cols[q] < need:
                c0 = issued_cols[q]
                ncols = int(min(CALL_COLS, stream_total[q] - c0))
                # cols c0..c0+ncols of stream q: find (block, j) per col
                # stream col x of chunk q = block bb, j = x - blk_start[bb]
                bb = int(np.searchsorted(blk_start[:, q], c0, side="right")) - 1
                take = 0
                while take < ncols:
                    xc = c0 + take
                    while (bb + 1 < BLOCKS and blk_start[bb + 1, q] <= xc):
                        bb += 1
                    j0 = xc - int(blk_start[bb, q])
                    n = int(min(ncols - take, Kbq[bb, q] - j0))
                    if n <= 0:  # zero-K blocks
                        bb += 1
                        continue
                    pieces = call_cols(bb, q, j0, n)
                    for c in range(N_CORES):
                        idx_cols[c].append(pieces[c])
                    take += n
                calls.append((q, ncols, img_off))
                img_off += ncols * 8
                issued_cols[q] += ncols

    # slab plan: consecutive CALLS_PER_SLAB calls -> one index DMA
    slab_plan = []  # (first_call, ncalls, img_off, img_cols)
    for s0 in range(0, len(calls), CALLS_PER_SLAB):
        cs = calls[s0:s0 + CALLS_PER_SLAB]
        o0 = cs[0][2]
        o1 = cs[-1][2] + cs[-1][1] * 8
        slab_plan.append((s0, len(cs), o0, o1 - o0))

    idx_in = [np.ascontiguousarray(np.concatenate(idx_cols[c], axis=1))
              for c in range(N_CORES)]
    assert idx_in[0].shape == (P, img_off), (idx_in[0].shape, img_off)
    col_off = img_off

    # --- per-core dense inputs
    xs = (np.asarray(x, np.float32) * dinv[:, None]).astype(BF16)
    in_maps = []
    w1_in = np.ascontiguousarray(
        np.asarray(W1).astype(BF16).reshape(NCHUNK, P, HID).transpose(1, 0, 2).reshape(P, NCHUNK * HID)
    )
    w2_in = np.ascontiguousarray(
        np.asarray(W2).astype(BF16).reshape(2, P, HID).transpose(1, 0, 2).reshape(P, 2 * HID)
    )
    wfc_in = np.ascontiguousarray(
        np.asarray(Wfc).astype(BF16).reshape(2, P, NCLS).transpose(1, 0, 2).reshape(P, 2 * NCLS)
    )
    has_b1 = bool(np.any(b1)) ; has_b2 = bool(np.any(b2)) ; has_bfc = bool(np.any(bfc))
    b1_in = np.tile(np.asarray(b1, np.float32)[None, :], (P, 1))
    b2_in = np.tile(np.asarray(b2, np.float32)[None, :], (P, 1))
    bfc_in = np.tile(np.asarray(bfc, np.float32)[None, :], (P, 1))

    for c in range(N_CORES):
        A = np.zeros((SLOTS, IN_DIM), BF16)
        A[:n_core[c]] = xs[ids_order[c]]
        xtt = np.ascontiguousarray(
            A.reshape(BLOCKS, P, NCHUNK, P).transpose(0, 3, 2, 1).reshape(BLOCKS, P, IN_DIM)
        )
        dv = np.ones(SLOTS, np.float32)
        dv[:n_core[c]] = dinv[ids_order[c]]
        dvp = np.ascontiguousarray(dv.reshape(BLOCKS, P).T)    # [128, 98]
        dv2p = np.ascontiguousarray((dv * dv).reshape(BLOCKS, P).T)
        m = {
            "xtt": xtt,
            "dinvp": dvp,
            "dinv2p": dv2p,
            "idxw": idx_in[c],
            "w1": w1_in,
            "w2": w2_in,
            "wfc": wfc_in,
        }
        if has_b1:
            m["b1b"] = b1_in
        if has_b2:
            m["b2b"] = b2_in
        if has_bfc:
            m["bfcb"] = bfc_in
        in_maps.append(m)

    plan = {
        "calls": calls,
        "block_first_call": block_first_call,
        "blk_start": blk_start,
        "Kbq": Kbq,
        "slab_plan": slab_plan,
        "idx_cols_total": col_off,
        "has_b1": has_b1,
        "has_b2": has_b2,
        "has_bfc": has_bfc,
        "inflation": plan_inflation,
        "n_core": n_core,
    }
    return plan, in_maps, ids_order


def _build_program(plan, sim_single_core=False, stop_after="full", dbg=False):
    """Build the SPMD Bass program (one program, all cores).

    stop_after: one of "mm1", "ag1", "g1", "ag2", "g2", "full" - truncates
    the program after that phase (g1 includes the fused mm2; g2 includes
    the fused FC head, so "g2" == "full")."""
    STAGES = ["mm1", "ag1", "g1", "ag2", "g2", "full"]
    stop_idx = STAGES.index(stop_after)
    nc = bacc.Bacc("TRN2", target_bir_lowering=False, debug=False,
                   num_devices=N_CORES, dynamic_dma_scratch_size=32768)
    dt = mybir.dt

    OFF = plan["idx_cols_total"]
    xtt = nc.dram_tensor("xtt", [BLOCKS, P, IN_DIM], dt.bfloat16, kind="ExternalInput")
    dinvp = nc.dram_tensor("dinvp", [P, BLOCKS], dt.float32, kind="ExternalInput")
    dinv2p = nc.dram_tensor("dinv2p", [P, BLOCKS], dt.float32, kind="ExternalInput")
    idxw = nc.dram_tensor("idxw", [P, OFF], dt.int16, kind="ExternalInput")
    w1 = nc.dram_tensor("w1", [P, NCHUNK * HID], dt.bfloat16, kind="ExternalInput")
    w2 = nc.dram_tensor("w2", [P, 2 * HID], dt.bfloat16, kind="ExternalInput")
    wfc = nc.dram_tensor("wfc", [P, 2 * NCLS], dt.bfloat16, kind="ExternalInput")
    b1b = (nc.dram_tensor("b1b", [P, HID], dt.float32, kind="ExternalInput")
           if plan["has_b1"] else None)
    b2b = (nc.dram_tensor("b2b", [P, HID], dt.float32, kind="ExternalInput")
           if plan["has_b2"] else None)
    bfcb = (nc.dram_tensor("bfcb", [P, NCLS], dt.float32, kind="ExternalInput")
            if plan["has_bfc"] else None)
    out = nc.dram_tensor("out", [SLOTS, NCLS], dt.bfloat16, kind="ExternalOutput")

    dbg_kind = {"kind": "ExternalOutput"} if dbg else {}
    hloc1 = nc.dram_tensor("hloc1", [SLOTS, HID], dt.bfloat16, **dbg_kind)
    hloc2 = nc.dram_tensor("hloc2", [SLOTS, HID], dt.bfloat16, **dbg_kind)
    hfull1 = nc.dram_tensor("hfull1", [N_CORES * SLOTS, HID], dt.bfloat16,
                            addr_space="Shared")
    hfull2 = nc.dram_tensor("hfull2", [N_CORES * SLOTS, HID], dt.bfloat16,
                            addr_space="Shared")

    calls = plan["calls"]
    block_first_call = plan["block_first_call"]
    blk_start = plan["blk_start"]
    Kbq = plan["Kbq"]
    slab_plan = plan["slab_plan"]
    max_slab_cols = max(sp[3] for sp in slab_plan)
    call_slab = np.zeros(len(calls), np.int64)
    for s, (fc, ncs, _, _) in enumerate(slab_plan):
        call_slab[fc:fc + ncs] = s
    stream_call = [[] for _ in range(NCHUNK)]
    for k, (q, ncols, ioff) in enumerate(calls):
        for cc in range(ncols):
            stream_call[q].append((k, cc))

    with tile.TileContext(nc) as tc:
        with (
            tc.tile_pool(name="const", bufs=1) as constp,
            tc.tile_pool(name="hl", bufs=1) as hlp,
            tc.tile_pool(name="xt", bufs=3) as xtp,
            tc.tile_pool(name="idx", bufs=2) as idxp,
            tc.tile_pool(name="g", bufs=12) as gp,
            tc.tile_pool(name="hsmall", bufs=3) as hsp,
            tc.tile_pool(name="fco", bufs=2) as fcop,
            tc.tile_pool(name="aggps", bufs=2, space="PSUM") as aggps,
            tc.tile_pool(name="mmps", bufs=2, space="PSUM") as mmps,
            tc.tile_pool(name="tpps", bufs=2, space="PSUM") as tpps,
            tc.tile_pool(name="fcps", bufs=2, space="PSUM") as fcps,
        ):
            # resident constants
            w1_sb = constp.tile([P, NCHUNK * HID], dt.bfloat16)
            nc.sync.dma_start(out=w1_sb[:], in_=w1[:])
            w2_sb = constp.tile([P, 2 * HID], dt.bfloat16)
            nc.sync.dma_start(out=w2_sb[:], in_=w2[:])
            wfc_sb = constp.tile([P, 2 * NCLS], dt.bfloat16)
            nc.sync.dma_start(out=wfc_sb[:], in_=wfc[:])
            dv_sb = constp.tile([P, BLOCKS], dt.float32)
            nc.sync.dma_start(out=dv_sb[:], in_=dinvp[:])
            dv2_sb = constp.tile([P, BLOCKS], dt.float32)
            nc.sync.dma_start(out=dv2_sb[:], in_=dinv2p[:])
            ident = constp.tile([P, P], dt.bfloat16)
            make_identity(nc, ident[:])
            b1_sb = b2_sb = bfc_sb = None
            if b1b is not None:
                b1_sb = constp.tile([P, HID], dt.float32)
                nc.sync.dma_start(out=b1_sb[:], in_=b1b[:])
            if b2b is not None:
                b2_sb = constp.tile([P, HID], dt.float32)
                nc.sync.dma_start(out=b2_sb[:], in_=b2b[:])
            if bfcb is not None:
                bfc_sb = constp.tile([P, NCLS], dt.float32)
                nc.sync.dma_start(out=bfc_sb[:], in_=bfcb[:])

            # per-group local table rows (dinv-scaled), SBUF-resident;
            # hl_view(b) is block b's [P, HID] slice.
            hlg = []
            for g_ in range(BLOCKS // MMG):
                hl_g = hlp.tile([P, MMG * HID], dt.bfloat16, tag=f"hl{g_}",
                                name=f"hl{g_}")
                hlg.append(hl_g)

            def hl_view(b):
                return hlg[b // MMG][:, (b % MMG) * HID:(b % MMG + 1) * HID]

            # ---- mm1: hl[b] = (dinv*x @ W1) block rows; also -> hloc1
            for g_ in range(BLOCKS // MMG):
                at = xtp.tile([P, MMG * IN_DIM], dt.bfloat16, tag="xt")
                nc.sync.dma_start(
                    out=at[:].rearrange("p (n f) -> p n f", f=IN_DIM),
                    in_=xtt[g_ * MMG:(g_ + 1) * MMG].rearrange(
                        "n p f -> p n f"),
                )
                for j in range(MMG):
                    mb = g_ * MMG + j
                    ps = mmps.tile([P, HID], dt.float32, space="PSUM", tag="mm")
                    for k in range(NCHUNK):
                        nc.tensor.matmul(
                            out=ps[:],
                            lhsT=at[:, j * IN_DIM + k * P:j * IN_DIM + (k + 1) * P],
                            rhs=w1_sb[:, k * HID:(k + 1) * HID],
                            start=(k == 0),
                            stop=(k == NCHUNK - 1),
                        )
                    nc.scalar.copy(out=hl_view(mb), in_=ps[:])
                nc.sync.dma_start(
                    out=hloc1[g_ * MMG * P:(g_ + 1) * MMG * P, :].rearrange(
                        "(n p) f -> p n f", p=P),
                    in_=hlg[g_][:].rearrange("p (n f) -> p n f", f=HID),
                )

            def all_gather(hloc, hfull):
                if sim_single_core:
                    nc.sync.dma_start(out=hfull[0:SLOTS, :], in_=hloc[:])
                else:
                    nc.gpsimd.collective_compute(
                        "AllGather",
                        mybir.AluOpType.bypass,
                        replica_groups=[list(range(N_CORES))],
                        ins=[hloc[:]],
                        outs=[hfull[:]],
                    )

            class GatherState:
                """Per-layer streaming state: issued calls, live call tiles,
                the current index slab."""

                def __init__(self, hfull):
                    self.hfull = hfull
                    self.n_issued = 0
                    self.tiles = {}       # call id -> (tile, ncols)
                    self.slab = None      # (slab id, tile)

                def issue(self, k):
                    (q, ncols, ioff) = calls[k]
                    s = int(call_slab[k])
                    if self.slab is None or self.slab[0] != s:
                        it = idxp.tile([P, max_slab_cols], dt.int16, tag="idx")
                        (fc, ncs, o0, nclm) = slab_plan[s]
                        nc.sync.dma_start(out=it[:, :nclm],
                                          in_=idxw[:, o0:o0 + nclm])
                        self.slab = (s, it)
                    (fc, ncs, o0, nclm) = slab_plan[s]
                    it = self.slab[1]
                    g = gp.tile([P, CALL_COLS * HID], dt.bfloat16, tag="g")
                    nidx = P * ncols
                    nc.gpsimd.dma_gather(
                        g[:, :ncols * HID].rearrange("p (l d) -> p l d", d=HID),
                        self.hfull[q * CH_ROWS:(q + 1) * CH_ROWS, :],
                        it[:, ioff - o0:ioff - o0 + ncols * 8],
                        nidx,
                        nidx,
                        HID,
                    )
                    self.tiles[k] = (g, ncols)
                    self.tiles.pop(k - 64, None)
                    self.n_issued = k + 1

            def gather_block(b, st, agg):
                """PSUM agg accumulates all gathered messages + the local
                self row for block b (start on first matmul, no stop)."""
                # issue every call this block needs (stream prefix property:
                # all calls with id < block_first_call[b+1])
                hi = (block_first_call[b + 1] if b + 1 < BLOCKS
                      else len(calls))
                while st.n_issued < hi:
                    st.issue(st.n_issued)
                first = True
                for q in range(NCHUNK):
                    sc = int(blk_start[b, q])
                    for j in range(int(Kbq[b, q])):
                        x = sc + j
                        k_id, cc = stream_call[q][x]
                        g, _nc = st.tiles[k_id]
                        nc.tensor.matmul(
                            out=agg[:],
                            lhsT=ident[:],
                            rhs=g[:, cc * HID:(cc + 1) * HID],
                            start=first,
                            stop=False,
                        )
                        first = False
                # self-loop message = local table row
                nc.tensor.matmul(
                    out=agg[:], lhsT=ident[:], rhs=hl_view(b),
                    start=first, stop=True,
                )

            def transpose2(h, tag):
                """h [128 nodes, 256 feat] -> ht [128 feat(k-part), 256]."""
                ht = hsp.tile([P, HID], dt.bfloat16, tag=tag)
                for k in range(2):
                    tp = tpps.tile([P, P], dt.bfloat16, space="PSUM", tag="tp")
                    nc.tensor.transpose(
                        out=tp[:], in_=h[:, k * P:(k + 1) * P], identity=ident[:]
                    )
                    nc.scalar.copy(out=ht[:, k * P:(k + 1) * P], in_=tp[:])
                return ht

            def act_store(agg, scale_ap, b_sb, h, func=mybir.ActivationFunctionType.Relu):
                """h = relu(scale*agg [+ scale_row*b]) from PSUM."""
                if b_sb is None:
                    nc.scalar.activation(out=h[:], in_=agg[:], func=func,
                                         scale=scale_ap)
                else:
                    hf = hsp.tile([P, HID], dt.float32, tag="hf")
                    nc.vector.tensor_scalar(
                        out=hf[:], in0=agg[:], scalar1=scale_ap, scalar2=None,
                        op0=mybir.AluOpType.mult,
                    )
                    nc.vector.tensor_tensor(
                        out=hf[:], in0=hf[:], in1=b_sb[:],
                        op=mybir.AluOpType.add,
                    )
                    nc.scalar.activation(out=h[:], in_=hf[:], func=func)

            # ---- layer 1 AllGather + gather/aggregate (+ fused mm2)
            if stop_idx >= 1:
                all_gather(hloc1, hfull1)
            if stop_idx >= 2:
                # bias algebra: h1s = relu(dv2*agg + dv*b1); dv*b1 is built
                # per block as an outer product on DVE when b1 != 0.
                st1 = GatherState(hfull1)
                for b in range(BLOCKS):
                    agg = aggps.tile([P, HID], dt.float32, space="PSUM",
                                     tag="agg")
                    gather_block(b, st1, agg)
                    h1s = hsp.tile([P, HID], dt.bfloat16, tag="h")
                    if b1_sb is None:
                        act_store(agg, dv2_sb[:, b:b + 1], None, h1s)
                    else:
                        hf = hsp.tile([P, HID], dt.float32, tag="hf")
                        nc.vector.tensor_scalar(
                            out=hf[:], in0=agg[:],
                            scalar1=dv2_sb[:, b:b + 1], scalar2=None,
                            op0=mybir.AluOpType.mult,
                        )
                        bs = hsp.tile([P, HID], dt.float32, tag="bs")
                        nc.vector.tensor_scalar(
                            out=bs[:], in0=b1_sb[:],
                            scalar1=dv_sb[:, b:b + 1], scalar2=None,
                            op0=mybir.AluOpType.mult,
                        )
                        nc.vector.tensor_tensor(
                            out=hf[:], in0=hf[:], in1=bs[:],
                            op=mybir.AluOpType.add,
                        )
                        nc.scalar.activation(
                            out=h1s[:], in_=hf[:],
                            func=mybir.ActivationFunctionType.Relu,
                        )
                    # fused mm2: hl[b] <- (h1s @ W2) (rows already carry
                    # dinv via h1s = dinv*relu(...)).
                    ht = transpose2(h1s, "ht")
                    ps2 = mmps.tile([P, HID], dt.float32, space="PSUM",
                                    tag="mm")
                    for k in range(2):
                        nc.tensor.matmul(
                            out=ps2[:],
                            lhsT=ht[:, k * P:(k + 1) * P],
                            rhs=w2_sb[:, k * HID:(k + 1) * HID],
                            start=(k == 0),
                            stop=(k == 1),
                        )
                    nc.scalar.copy(out=hl_view(b), in_=ps2[:])
                    if (b + 1) % MMG == 0:
                        g_ = b // MMG
                        nc.sync.dma_start(
                            out=hloc2[g_ * MMG * P:(g_ + 1) * MMG * P, :]
                            .rearrange("(n p) f -> p n f", p=P),
                            in_=hlg[g_][:].rearrange("p (n f) -> p n f",
                                                     f=HID),
                        )

            # ---- layer 2 AllGather + gather/aggregate (+ fused FC head)
            if stop_idx >= 3:
                all_gather(hloc2, hfull2)
            if stop_idx >= 4:
                st2 = GatherState(hfull2)
                for b in range(BLOCKS):
                    agg = aggps.tile([P, HID], dt.float32, space="PSUM",
                                     tag="agg")
                    gather_block(b, st2, agg)
                    h2 = hsp.tile([P, HID], dt.bfloat16, tag="h")
                    act_store(agg, dv_sb[:, b:b + 1], b2_sb, h2)
                    ht = transpose2(h2, "ht")
                    fo = fcop.tile([P, NCLS], dt.bfloat16, tag="fco")
                    for n in range(2):
                        ps = fcps.tile([P, NCLS // 2], dt.float32,
                                       space="PSUM", tag="fc")
                        for k in range(2):
                            nc.tensor.matmul(
                                out=ps[:],
                                lhsT=ht[:, k * P:(k + 1) * P],
                                rhs=wfc_sb[:, k * NCLS + n * (NCLS // 2):
                                           k * NCLS + (n + 1) * (NCLS // 2)],
                                start=(k == 0),
                                stop=(k == 1),
                            )
                        if bfc_sb is not None:
                            hf = fcop.tile([P, NCLS // 2], dt.float32,
                                           tag="fcf")
                            nc.vector.tensor_tensor(
                                out=hf[:], in0=ps[:],
                                in1=bfc_sb[:, n * (NCLS // 2):
                                           (n + 1) * (NCLS // 2)],
                                op=mybir.AluOpType.add,
                            )
                            nc.vector.tensor_copy(
                                out=fo[:, n * (NCLS // 2):(n + 1) * (NCLS // 2)],
                                in_=hf[:],
                            )
                        else:
                            nc.vector.tensor_copy(
                                out=fo[:, n * (NCLS // 2):(n + 1) * (NCLS // 2)],
                                in_=ps[:],
                            )
                    nc.sync.dma_start(out=out[b * P:(b + 1) * P, :],
                                      in_=fo[:])

    nc.compile()
    return nc


_CACHE = {}


def kernel(x, edge_index, W1, b1, W2, b2, Wfc, bfc):
    x = np.asarray(x)
    plan, in_maps, ids_order = _preprocess(x, edge_index, W1, b1, W2, b2, Wfc, bfc)
    nc = _build_program(plan)
    res = run_bass_kernel_spmd(nc, in_maps, core_ids=list(range(N_CORES)))
    full = np.empty((N_NODES, NCLS), np.float32)
    for c in range(N_CORES):
        full[ids_order[c]] = res.results[c]["out"][: len(ids_order[c])].astype(
            np.float32)
    return full
